# revision 36
# baseline (speedup 1.0000x reference)
"""Trainium2 Bass kernel for GQA attention (B=2, S=2048, D=1024, H=16, HKV=4).

Sharding: 8 cores = batch(2) x kv-group(4). Each core handles one batch and
one KV head group (4 query heads + 1 KV head), computes attention plus its
partial slice of the output projection (row-parallel wo); the host sums the
4 tensor-parallel partials per batch (partials are bf16, summed in f32).
No device collectives.

Per-core device kernel (matmul operands bf16, fp32 PSUM accumulation):
  1. QKV projection with xT chunks stationary, natural-layout output,
     emitted in 4 groups of 4 s-blocks.
  2. RoPE on q,k on DVE, 12 ops per group. Weight rows are pre-permuted on
     host so each head's dims are [32 real | 32 imag] (contiguous slices).
  3. PE-transpose roped q,k -> qT [hd,s]; kT duplicated to partitions
     64-127 so even/odd heads' score matmuls run concurrently via PE row
     tiling.
  4. Attention in two q-column half-passes per head pair. Per 128-row
     k-block: S^T = kT.T @ qT for both heads into one [128,1024] PSUM tile
     (two banks -> concurrent matmuls); diagonal blocks masked by
     accumulating identity @ (-1e9 upper-triangle) on top; causal upper
     blocks never computed. One exp per chunk covers both heads (ACT,
     scale=1/8 folded in; max-subtraction skipped - scores are O(1) here
     and masked entries come in as exp(-1e9)=0).
  5. PV with a ones-row appended to V accumulates out^T[d,q] per half-pass
     in PSUM ([65,1024] per head); row 64 collects the softmax denominator
     for free. PV lags scores by one k-block so PE never waits on ACT.
  6. Normalize via DVE reciprocal + gpsimd partition-broadcast + multiply;
     odd head moved into place with one SBUF->SBUF DMA.
  7. y_partial = attn^T.T @ woT; first half of the s-blocks is emitted
     between pair-1's last PV and its normalize to hide the tail.
  Pipeline: both pairs' pass-1 scores are prefetched into a transient PSUM
  pool between stage-1 groups, and pair-0 pass-2 score backlog is queued
  around the PE-only PV blocks, so ACT (the bottleneck engine, ~70us of
  irreducible exp work) stays fed end to end.
"""

import numpy as np
import ml_dtypes

B, S, D = 2, 2048, 1024
H, HKV, HD = 16, 4, 64
REP = H // HKV  # 4 query heads per kv head
N_CORES = 8
NSB = S // 128  # 16 s-blocks
NDC = D // 128  # 8 d-chunks
QKV = REP * HD + 2 * HD  # 384 projected dims per core
BF16 = ml_dtypes.bfloat16

# col offsets of each kb's exp-score span (per head) inside eSall
OFF = [0]
for _kb in range(NSB):
    OFF.append(OFF[-1] + (S - 128 * _kb))
TOT = OFF[-1]  # 17408

_CACHE = {}


def _chunks(kb):
    """512-aligned score chunks for k-block kb: (gs, ge) in global q cols."""
    out = []
    for qc in range(kb // 4, 4):
        gs = max(qc * 512, kb * 128)
        ge = (qc + 1) * 512
        out.append((gs, ge))
    return out


def _build_module():
    from contextlib import ExitStack

    import concourse.bacc as bacc
    import concourse.mybir as mybir
    import concourse.tile as tile
    from concourse.alu_op_type import AluOpType

    f32 = mybir.dt.float32
    bf16 = mybir.dt.bfloat16
    Exp = mybir.ActivationFunctionType.Exp
    mult, add, sub = AluOpType.mult, AluOpType.add, AluOpType.subtract

    nc = bacc.Bacc("TRN2", target_bir_lowering=False, debug=False,
                   num_devices=N_CORES)

    xT_d = nc.dram_tensor("xT", (D, S), bf16, kind="ExternalInput").ap()
    wcat_d = nc.dram_tensor("wcatT", (D, QKV), bf16, kind="ExternalInput").ap()
    woT_d = nc.dram_tensor("woT", (2 * 128, D), bf16, kind="ExternalInput").ap()
    ctq_d = nc.dram_tensor("ctq", (128, S), bf16, kind="ExternalInput").ap()
    stq_d = nc.dram_tensor("stq", (128, S), bf16, kind="ExternalInput").ap()
    ctk_d = nc.dram_tensor("ctk", (128, NSB * 32), bf16, kind="ExternalInput").ap()
    stk_d = nc.dram_tensor("stk", (128, NSB * 32), bf16, kind="ExternalInput").ap()
    mneg_d = nc.dram_tensor("maskneg", (128, 128), bf16, kind="ExternalInput").ap()
    idn_d = nc.dram_tensor("ident", (128, 128), bf16, kind="ExternalInput").ap()
    y_d = nc.dram_tensor("y", (S, D), bf16, kind="ExternalOutput").ap()

    with tile.TileContext(nc) as tc:
        with ExitStack() as ctx:
            persist = ctx.enter_context(tc.tile_pool(name="persist", bufs=1))
            xT = persist.tile([128, NDC * S], bf16)      # 32 KB/part
            wcat = persist.tile([128, NDC * QKV], bf16)  # 6 KB
            woT = persist.tile([128, 2 * D], bf16)       # 4 KB
            ctq = persist.tile([128, S], bf16)
            stq = persist.tile([128, S], bf16)
            ctk = persist.tile([128, NSB * 32], bf16)
            stk = persist.tile([128, NSB * 32], bf16)
            mneg = persist.tile([128, 128], bf16)
            idn = persist.tile([128, 128], bf16)
            qkv = persist.tile([128, NSB * 320], bf16)   # 10 KB (q+k natural)
            qT = persist.tile([128, 2 * S], bf16)        # 8 KB
            kT2 = persist.tile([128, S], bf16)           # 4 KB
            vb = persist.tile([128, NSB * 65], bf16)     # ~2 KB (v + ones col)
            attnT0 = persist.tile([128, S], bf16, tag="attnT0")
            attnT1 = persist.tile([128, S], bf16, tag="attnT1")
            attnT = [attnT0, attnT1]
            dummy = persist.tile([1, 8], f32)

            for dc in range(NDC):
                nc.sync.dma_start(xT[:, dc * S:(dc + 1) * S],
                                  xT_d[dc * 128:(dc + 1) * 128, :])
                nc.sync.dma_start(wcat[:, dc * QKV:(dc + 1) * QKV],
                                  wcat_d[dc * 128:(dc + 1) * 128, :])
            nc.sync.dma_start(ctq[:], ctq_d[:])
            nc.sync.dma_start(stq[:], stq_d[:])
            nc.sync.dma_start(ctk[:], ctk_d[:])
            nc.sync.dma_start(stk[:], stk_d[:])
            nc.sync.dma_start(idn[:], idn_d[:])
            nc.sync.dma_start(mneg[:], mneg_d[:])
            for c in range(2):
                nc.sync.dma_start(woT[:, c * D:(c + 1) * D],
                                  woT_d[c * 128:(c + 1) * 128, :])
            nc.gpsimd.memset(vb[:], 1.0)
            # warm the ACT exp table while DMAs run
            nc.gpsimd.memset(dummy[:], 0.0)
            nc.scalar.activation(dummy[:], dummy[:], Exp)

            s1 = ctx.enter_context(ExitStack())
            psq = s1.enter_context(
                tc.tile_pool(name="psqkv", bufs=2, space="PSUM"))
            tpq = s1.enter_context(
                tc.tile_pool(name="tpq", bufs=1, space="PSUM"))
            tpk = s1.enter_context(
                tc.tile_pool(name="tpk", bufs=1, space="PSUM"))
            tmp = ctx.enter_context(tc.tile_pool(name="ropetmp", bufs=2))
            recp = ctx.enter_context(tc.tile_pool(name="recip", bufs=2))
            rcbp = ctx.enter_context(tc.tile_pool(name="rcbpool", bufs=2))
            otp = ctx.enter_context(tc.tile_pool(name="oddtmp", bufs=2))
            yst = ctx.enter_context(tc.tile_pool(name="ystage", bufs=3))

            # ---- stage 1 (per 4-s-block group): proj + rope + transposes --
            # qkv layout: col = sb*320 + h*64 + half*32 + j  (q)
            #             col = sb*320 + 256 + half*32 + j   (k)
            def emit_group(g, on_act=True):
                for sbl in range(4):
                    sb = 4 * g + sbl
                    ps = psq.tile([128, QKV], f32, tag="ps", name=f"ps{sb}")
                    for dc in range(NDC):
                        nc.tensor.matmul(
                            ps[:],
                            lhsT=xT[:, dc * S + sb * 128:
                                    dc * S + (sb + 1) * 128],
                            rhs=wcat[:, dc * QKV:(dc + 1) * QKV],
                            start=(dc == 0), stop=(dc == NDC - 1))
                    if on_act:
                        nc.scalar.copy(
                            qkv[:, sb * 320: sb * 320 + 320], ps[:, 0:320])
                        nc.scalar.copy(
                            vb[:, sb * 65: sb * 65 + 64], ps[:, 320:384])
                    else:
                        nc.vector.tensor_copy(
                            qkv[:, sb * 320: sb * 320 + 320], ps[:, 0:320])
                        nc.vector.tensor_copy(
                            vb[:, sb * 65: sb * 65 + 64], ps[:, 320:384])

                # rope for group g (4 s-blocks at once), in place
                base = g * 1280
                g4 = qkv[:, base:base + 1280].rearrange(
                    "p (sbl x) -> p sbl x", sbl=4)
                qg = g4[:, :, 0:256].rearrange("p sbl (h c) -> p sbl h c",
                                               c=64)
                qr, qi = qg[:, :, :, 0:32], qg[:, :, :, 32:64]
                kg = g4[:, :, 256:320]
                kr, ki = kg[:, :, 0:32], kg[:, :, 32:64]
                ct = ctq[:, g * 512:(g + 1) * 512].rearrange(
                    "p (sbl h j) -> p sbl h j", sbl=4, h=REP)
                st = stq[:, g * 512:(g + 1) * 512].rearrange(
                    "p (sbl h j) -> p sbl h j", sbl=4, h=REP)
                ctks = ctk[:, g * 128:(g + 1) * 128].rearrange(
                    "p (sbl j) -> p sbl j", sbl=4)
                stks = stk[:, g * 128:(g + 1) * 128].rearrange(
                    "p (sbl j) -> p sbl j", sbl=4)
                tA = tmp.tile([128, 512], bf16, tag="tA", name=f"tA{g}")
                tB = tmp.tile([128, 512], bf16, tag="tB", name=f"tB{g}")
                tC = tmp.tile([128, 512], bf16, tag="tC", name=f"tC{g}")
                tD = tmp.tile([128, 512], bf16, tag="tD", name=f"tD{g}")
                r3 = lambda t: t[:].rearrange("p (sbl h j) -> p sbl h j",
                                              sbl=4, h=REP)
                nc.vector.tensor_tensor(r3(tA), qr, ct, mult)
                nc.vector.tensor_tensor(r3(tB), qi, st, mult)
                nc.vector.tensor_tensor(r3(tC), qr, st, mult)
                nc.vector.tensor_tensor(r3(tD), qi, ct, mult)
                nc.vector.tensor_tensor(qr, r3(tA), r3(tB), sub)
                nc.vector.tensor_tensor(qi, r3(tC), r3(tD), add)
                tE = tmp.tile([128, 128], bf16, tag="tE", name=f"tE{g}")
                tF = tmp.tile([128, 128], bf16, tag="tF", name=f"tF{g}")
                tG = tmp.tile([128, 128], bf16, tag="tG", name=f"tG{g}")
                tH = tmp.tile([128, 128], bf16, tag="tH", name=f"tH{g}")
                r2 = lambda t: t[:].rearrange("p (sbl j) -> p sbl j", sbl=4)
                nc.vector.tensor_tensor(r2(tE), kr, ctks, mult)
                nc.vector.tensor_tensor(r2(tF), ki, stks, mult)
                nc.vector.tensor_tensor(r2(tG), kr, stks, mult)
                nc.vector.tensor_tensor(r2(tH), ki, ctks, mult)
                nc.vector.tensor_tensor(kr, r2(tE), r2(tF), sub)
                nc.vector.tensor_tensor(ki, r2(tG), r2(tH), add)

                # transposes: q -> qT, k -> kT2[0:64]
                for half in range(2):  # sbl pairs (0,1) and (2,3)
                    pt = tpq.tile([128, 512], bf16, tag="ptq",
                                  name=f"ptq{g}_{half}")
                    for li, (sbl, hb) in enumerate(
                            [(2 * half, 0), (2 * half, 1),
                             (2 * half + 1, 0), (2 * half + 1, 1)]):
                        src = qkv[:, base + sbl * 320 + hb * 128:
                                  base + sbl * 320 + hb * 128 + 128]
                        nc.tensor.transpose(
                            pt[:, li * 128:(li + 1) * 128], src, idn[:])
                    dst = qT[:].rearrange(
                        "p (hb sb c) -> p sb hb c", hb=2, sb=NSB)[
                        :, 4 * g + 2 * half: 4 * g + 2 * half + 2, :, :]
                    nc.vector.tensor_copy(
                        dst, pt[:].rearrange("p (sb hb c) -> p sb hb c",
                                             sb=2, hb=2))
                ptk = tpk.tile([64, 512], bf16, tag="ptk", name=f"ptk{g}")
                for sbl in range(4):
                    nc.tensor.transpose(
                        ptk[:, sbl * 128:(sbl + 1) * 128],
                        qkv[:, base + sbl * 320 + 256:
                            base + sbl * 320 + 320],
                        idn[:])
                nc.vector.tensor_copy(
                    kT2[0:64, g * 512:(g + 1) * 512], ptk[:])
                nc.sync.dma_start(kT2[64:128, g * 512:(g + 1) * 512],
                                  kT2[0:64, g * 512:(g + 1) * 512])

            # ---- stage 2: attention, two q-column half-passes per pair --
            PASS_KBS = [list(range(8)), list(range(NSB))]
            EOFF = []
            ESZ = 0
            for h2 in range(2):
                off = {}
                cum = 0
                for kb in PASS_KBS[h2]:
                    lo_k = max(kb * 128, h2 * 1024)
                    off[kb] = cum
                    cum += 2 * ((h2 + 1) * 1024 - lo_k)
                EOFF.append(off)
                ESZ = max(ESZ, cum)
            eSb = persist.tile([128, ESZ], bf16, name="eSb")
            pools = {}

            def normalize(po, hp, h2, ihead, lo=0, w=1024):
                c0 = h2 * 1024 + lo
                rc = recp.tile([1, 1024], f32, tag="rc",
                               name=f"rc{hp}_{h2}_{ihead}_{lo}")
                nc.vector.reciprocal(rc[0:1, 0:w], po[64:65, lo:lo + w])
                rcb = rcbp.tile([64, 1024], f32, tag="rcb",
                                name=f"rcb{hp}_{h2}_{ihead}_{lo}")
                nc.gpsimd.partition_broadcast(rcb[0:64, 0:w], rc[0:1, 0:w])
                if ihead == 0:
                    nc.vector.tensor_tensor(
                        attnT[hp][0:64, c0:c0 + w],
                        po[0:64, lo:lo + w], rcb[0:64, 0:w], mult)
                else:
                    ot = otp.tile([64, 1024], bf16, tag="ot",
                                  name=f"ot{hp}_{h2}_{lo}")
                    nc.vector.tensor_tensor(ot[0:64, 0:w],
                                            po[0:64, lo:lo + w],
                                            rcb[0:64, 0:w], mult)
                    nc.sync.dma_start(attnT[hp][64:128, c0:c0 + w],
                                      ot[0:64, 0:w])

            def scores_chunks(h2, kb):
                lo_k = max(kb * 128, h2 * 1024)
                w = (h2 + 1) * 1024 - lo_k
                chunks = []
                for qc in range(max(2 * h2, kb // 4), 2 * h2 + 2):
                    gs = max(qc * 512, kb * 128)
                    chunks.append((gs, (qc + 1) * 512))
                return lo_k, w, chunks

            def emit_scores_kb(hp, h2, kb, pool, ebase=0):
                lo_k, w, chunks = scores_chunks(h2, kb)
                for (gs, ge) in chunks:
                    n = ge - gs
                    pp = pool.tile([128, 1024], f32, tag="pp",
                                   name=f"pp{hp}_{h2}_{kb}_{gs}")
                    diag = (gs == kb * 128)
                    for i in range(2):
                        nc.tensor.matmul(
                            pp[:, i * 512: i * 512 + n],
                            lhsT=kT2[i * 64:(i + 1) * 64,
                                     kb * 128:(kb + 1) * 128],
                            rhs=qT[i * 64:(i + 1) * 64,
                                   hp * S + gs: hp * S + ge],
                            start=True, stop=not diag)
                        if diag:
                            nc.tensor.matmul(
                                pp[:, i * 512: i * 512 + 128],
                                lhsT=idn[:], rhs=mneg[:],
                                start=False, stop=True,
                                skip_group_check=True)
                    src = pp[:].rearrange("p (h c) -> p h c", h=2)[:, :, 0:n]
                    lo = gs - lo_k
                    e0 = ebase + EOFF[h2][kb]
                    dst = eSb[:, e0: e0 + 2 * w].rearrange(
                        "p (h c) -> p h c", h=2)[:, :, lo:lo + n]
                    nc.scalar.activation(dst, src, Exp, scale=0.125)

            def alloc_po(hp, h2):
                return [pools["psO"].tile([65, 1024], f32, tag="po",
                                          name=f"po{hp}_{h2}_{i}")
                        for i in range(2)]

            def emit_pv_kb(po, hp, h2, kb, ebase=0):
                lo_k, w, chunks = scores_chunks(h2, kb)
                for (gs, ge) in chunks:
                    qc = ge // 512 - 1
                    for i in range(2):
                        eo = ebase + EOFF[h2][kb] + i * w + (gs - lo_k)
                        nc.tensor.matmul(
                            po[i][:, gs - h2 * 1024: ge - h2 * 1024],
                            lhsT=vb[:, kb * 65: kb * 65 + 65],
                            rhs=eSb[:, eo: eo + (ge - gs)],
                            start=(kb == 0),
                            stop=(kb == min(NSB - 1, 4 * qc + 3)))

            def emit_pass(hp, h2, tail_hook=None):
                """Fused: scores/exp with PV delayed one k-block."""
                po = alloc_po(hp, h2)
                kbs = PASS_KBS[h2]
                for kb in kbs:
                    emit_scores_kb(hp, h2, kb, pools["psS"])
                    if kb > kbs[0]:
                        emit_pv_kb(po, hp, h2, kb - 1)
                emit_pv_kb(po, hp, h2, kbs[-1])
                if tail_hook is not None:
                    tail_hook()
                for i in range(2):
                    normalize(po[i], hp, h2, i)

            def emit_stage3(sb_range, alt_pool=False):
                for sb in sb_range:
                    if alt_pool and sb % 2 == 1:
                        yp = pools["psO"].tile([128, D], f32, tag="po",
                                               name=f"yp{sb}")
                    else:
                        yp = pools["psS"].tile([128, D], f32, tag="pp",
                                               name=f"yp{sb}")
                    for hp in range(2):
                        for c2 in range(2):
                            nc.tensor.matmul(
                                yp[:, c2 * 512:(c2 + 1) * 512],
                                lhsT=attnT[hp][:, sb * 128:(sb + 1) * 128],
                                rhs=woT[:, hp * D + c2 * 512:
                                        hp * D + (c2 + 1) * 512],
                                start=(hp == 0), stop=(hp == 1))
                    ys = yst.tile([128, D], bf16, tag="ys", name=f"ys{sb}")
                    if sb % 2 == 0:
                        nc.scalar.copy(ys[:], yp[:])
                    else:
                        nc.vector.tensor_copy(ys[:], yp[:])
                    nc.sync.dma_start(y_d[sb * 128:(sb + 1) * 128, :],
                                      ys[:])

            emit_group(0, on_act=True)
            emit_group(1, on_act=True)
            # prefetch pair-0 pass-1 scores into a transient 4-bank pool
            # that coexists with the (slimmed) stage-1 pools
            lite = s1.enter_context(
                tc.tile_pool(name="lite", bufs=2, space="PSUM"))
            P1BASE = sum(2 * (1024 - 128 * kb) for kb in PASS_KBS[0])
            for kb in PASS_KBS[0]:
                emit_scores_kb(0, 0, kb, lite)
            for kb in PASS_KBS[0]:
                emit_scores_kb(1, 0, kb, lite, ebase=P1BASE)
            emit_group(2, on_act=False)
            emit_group(3, on_act=False)
            s1.close()
            pools["psS"] = ctx.enter_context(
                tc.tile_pool(name="psS", bufs=2, space="PSUM"))
            pools["psO"] = ctx.enter_context(
                tc.tile_pool(name="psO", bufs=2, space="PSUM"))
            po00 = alloc_po(0, 0)
            for kb in PASS_KBS[0]:
                emit_pv_kb(po00, 0, 0, kb)
            for i in range(2):
                normalize(po00[i], 0, 0, i)
            # queue exp backlog for ACT before the PE-only pv10 block
            for kb in range(3):
                emit_scores_kb(0, 1, kb, pools["psS"])
            po10 = alloc_po(1, 0)
            for kb in PASS_KBS[0]:
                emit_pv_kb(po10, 1, 0, kb, ebase=P1BASE)
            for i in range(2):
                normalize(po10[i], 1, 0, i)
            # rest of pair-0 pass-2, PV delayed one k-block
            po01 = alloc_po(0, 1)
            for kb in PASS_KBS[1]:
                if kb >= 3:
                    emit_scores_kb(0, 1, kb, pools["psS"])
                if kb >= 1:
                    emit_pv_kb(po01, 0, 1, kb - 1)
            emit_pv_kb(po01, 0, 1, PASS_KBS[1][-1])
            for i in range(2):
                normalize(po01[i], 0, 1, i)
            emit_pass(1, 1, tail_hook=lambda: emit_stage3(range(0, 8)))
            emit_stage3(range(8, NSB), alt_pool=True)

    nc.compile()
    return nc


def _get_module():
    if "nc" not in _CACHE:
        _CACHE["nc"] = _build_module()
    return _CACHE["nc"]


def _host_tables(freqs_cos, freqs_sin):
    # ctq[p, sb*128 + h*32 + j] = cos[sb*128 + p, j]  (tiled over 4 heads)
    c3 = freqs_cos.reshape(NSB, 128, 32).transpose(1, 0, 2)  # [p, sb, j]
    s3 = freqs_sin.reshape(NSB, 128, 32).transpose(1, 0, 2)
    ctq = np.broadcast_to(c3[:, :, None, :],
                          (128, NSB, REP, 32)).reshape(128, S)
    stq = np.broadcast_to(s3[:, :, None, :],
                          (128, NSB, REP, 32)).reshape(128, S)
    # ctk[p, sb*32 + j] = cos[sb*128 + p, j]
    ctk = np.ascontiguousarray(c3).reshape(128, NSB * 32)
    stk = np.ascontiguousarray(s3).reshape(128, NSB * 32)
    return ctq, stq, ctk, stk


def make_in_maps(x, wq, wk, wv, wo, freqs_cos, freqs_sin):
    x = np.asarray(x, np.float32)
    wq = np.asarray(wq, np.float32)
    wk = np.asarray(wk, np.float32)
    wv = np.asarray(wv, np.float32)
    wo = np.asarray(wo, np.float32)
    freqs_cos = np.asarray(freqs_cos, np.float32)
    freqs_sin = np.asarray(freqs_sin, np.float32)

    # deinterleave rope pairs within each head: [r0 i0 r1 i1 ...] ->
    # [r0..r31 | i0..i31]
    idx = np.concatenate([np.arange(0, HD, 2), np.arange(1, HD, 2)])
    wq_p = wq.reshape(H, HD, D)[:, idx, :].reshape(H * HD, D)
    wk_p = wk.reshape(HKV, HD, D)[:, idx, :].reshape(HKV * HD, D)

    ctq, stq, ctk, stk = _host_tables(freqs_cos, freqs_sin)
    kk, qq = np.arange(128)[:, None], np.arange(128)[None, :]
    maskneg = np.where(kk <= qq, 0.0, -1e9).astype(np.float32)
    ident = np.eye(128)

    common = {
        "ctq": ctq.astype(BF16), "stq": stq.astype(BF16),
        "ctk": ctk.astype(BF16), "stk": stk.astype(BF16),
        "maskneg": maskneg.astype(BF16), "ident": ident.astype(BF16),
    }
    xT_b = [np.ascontiguousarray(x[b].T).astype(BF16) for b in range(B)]
    in_maps = []
    for core in range(N_CORES):
        b, g = divmod(core, HKV)
        wqT = wq_p[g * 256:(g + 1) * 256].T
        wkT = wk_p[g * 64:(g + 1) * 64].T
        wvT = wv[g * 64:(g + 1) * 64].T
        wcat = np.ascontiguousarray(
            np.concatenate([wqT, wkT, wvT], axis=1)).astype(BF16)
        woTg = np.ascontiguousarray(wo[:, g * 256:(g + 1) * 256].T).astype(BF16)
        in_maps.append({"xT": xT_b[b], "wcatT": wcat, "woT": woTg, **common})
    return in_maps


def _causal_fast_path_ok(mask):
    m = np.asarray(mask)
    if m.shape != (S, S):
        return False
    upper = m[np.triu_indices(S, 1)]
    lower = m[np.tril_indices(S, 0)]
    return bool(np.all(upper <= -1e8) and np.all(lower == 0))


def _numpy_fallback(x, wq, wk, wv, wo, freqs_cos, freqs_sin, mask):
    x = np.asarray(x, np.float32)
    xq = (x.reshape(B * S, D) @ np.asarray(wq, np.float32).T).reshape(B, S, H, HD)
    xk = (x.reshape(B * S, D) @ np.asarray(wk, np.float32).T).reshape(B, S, HKV, HD)
    xv = (x.reshape(B * S, D) @ np.asarray(wv, np.float32).T).reshape(B, S, HKV, HD)

    def rope(t, nh):
        tf = t.reshape(B, S, nh, HD // 2, 2)
        tr, ti = tf[..., 0], tf[..., 1]
        c = np.asarray(freqs_cos, np.float32)[None, :, None, :]
        s = np.asarray(freqs_sin, np.float32)[None, :, None, :]
        outr = tr * c - ti * s
        outi = tr * s + ti * c
        return np.stack([outr, outi], axis=-1).reshape(B, S, nh, HD)

    xq = rope(xq, H)
    xk = rope(xk, HKV)
    xqg = xq.reshape(B, S, HKV, REP, HD)
    scores = np.einsum("bqgrd,bkgd->bgrqk", xqg, xk) / np.sqrt(np.float32(HD))
    scores = scores + np.asarray(mask, np.float32)[None, None, None, :, :]
    scores = scores - scores.max(axis=-1, keepdims=True)
    e = np.exp(scores)
    attn = e / e.sum(axis=-1, keepdims=True)
    out = np.einsum("bgrqk,bkgd->bqgrd", attn, xv).reshape(B, S, H * HD)
    return (out.reshape(B * S, H * HD) @ np.asarray(wo, np.float32)
            .T).reshape(B, S, D).astype(np.float32)


def kernel(x, wq, wk, wv, wo, freqs_cos, freqs_sin, mask):
    if not _causal_fast_path_ok(mask):
        return _numpy_fallback(x, wq, wk, wv, wo, freqs_cos, freqs_sin, mask)
    from concourse import bass_utils
    nc = _get_module()
    in_maps = make_in_maps(x, wq, wk, wv, wo, freqs_cos, freqs_sin)
    res = bass_utils.run_bass_kernel_spmd(nc, in_maps,
                                          core_ids=list(range(N_CORES)))
    y = np.zeros((B, S, D), np.float32)
    for core in range(N_CORES):
        b = core // HKV
        y[b] += res.results[core]["y"].astype(np.float32)
    return y
